# revision 1
# baseline (speedup 1.0000x reference)
"""Trainium2 Bass kernel for nn_BoundarySuppressionWithSmoothing.

Full inputs: x [8,1,512,1024] f32, prediction [8,1,512,1024] int32.
Sharding: pure data parallel, image i -> core i.

Per-core algorithm (image I [512,1024], layout A: 4 row-chunks of [128,1024]):
  - boundary detection via exp-encoded morphology on PE + ACT (exp/ln-free
    product compare), masks m3..m0 via a mask-carried dilation chain
  - 4 iterations of masked 3x3 box average with replication padding
  - separable dilated 7x7 Gaussian (dilation 6) via PE banded matmuls
"""
import math
import sys
from contextlib import ExitStack

import numpy as np

sys.path.insert(0, '/opt/trn_rl_repo')

import concourse.bass as bass  # noqa: E402
import concourse.bacc as bacc  # noqa: E402
import concourse.tile as tile  # noqa: E402
from concourse import mybir  # noqa: E402

P = 128
W = 1024
H = 512
CH = 4          # row chunks
B = 8           # batch == cores
ALPHA = 4.6     # morphology exp-encoding scale
PTHR = float(np.exp(4.2))   # product threshold for boundary test
DT = mybir.dt
AF = mybir.ActivationFunctionType
OP = mybir.AluOpType

USE_DIVIDE = True   # t = Y / n via TT divide; else reciprocal+mult


# ---------------------------------------------------------------- weights ---
def _gauss1d():
    size, sigma = 7, 1.0
    u = np.exp(-((np.arange(size) - 3.0) ** 2) / (2 * sigma ** 2))
    # 2D reference kernel is outer(u,u)/sum => separable 1D = u/sum(u)
    return (u / u.sum()).astype(np.float64)


def _round_fp32r(a):
    """Round fp32 array to fp32r (11 explicit mantissa bits) on host."""
    u = a.astype(np.float32).view(np.uint32).astype(np.uint64)
    u = (u + 0x800) & 0xFFFFF000
    return (u & 0xFFFFFFFF).astype(np.uint32).view(np.float32)


def build_host_consts():
    """All constant weight matrices, as one dict of fp32 arrays [128,x]."""
    c = {}
    tri = np.zeros((P, P), np.float32)
    for k in range(P):
        for d in (-1, 0, 1):
            if 0 <= k + d < P:
                tri[k, k + d] = 1.0   # lhsT[k,m]: out m from in k, |k-m|<=1
    c['T_mid'] = tri
    t_top = tri.copy(); t_top[0, 0] = 2.0
    c['T_top'] = t_top
    t_bot = tri.copy(); t_bot[P - 1, P - 1] = 2.0
    c['T_bot'] = t_bot
    t_up = np.zeros((P, P), np.float32); t_up[P - 1, 0] = 1.0
    c['T_up'] = t_up
    t_dn = np.zeros((P, P), np.float32); t_dn[0, P - 1] = 1.0
    c['T_dn'] = t_dn
    c['I'] = np.eye(P, dtype=np.float32)
    bvec = np.zeros((P, P), np.float32)
    bvec[:, 0] = -4.0; bvec[0, 0] = -3.0      # bv_top
    bvec[:, 1] = -4.0; bvec[P - 1, 1] = -3.0  # bv_bot
    c['BVEC'] = bvec

    g = _gauss1d()
    for j in range(7):
        c[f'G{j}'] = _round_fp32r(np.eye(P, dtype=np.float32) * g[j])
    # vertical gaussian: Wv[R,S] = sum_j g[j] [clamp(R+6(j-3),0,H-1)==S]
    Wv = np.zeros((H, H), np.float64)
    for R in range(H):
        for j in range(7):
            S = min(max(R + 6 * (j - 3), 0), H - 1)
            Wv[R, S] += g[j]
    for c_dst in range(CH):
        for c_src in range(CH):
            if abs(c_dst - c_src) > 1:
                continue
            blk = Wv[c_dst * P:(c_dst + 1) * P, c_src * P:(c_src + 1) * P]
            if not blk.any():
                continue
            # lhsT[k,m] = Wv[dst=128c+m, src=128c'+k]
            c[f'B_{c_dst}_{c_src}'] = _round_fp32r(
                np.ascontiguousarray(blk.T).astype(np.float32))
    return c


# ----------------------------------------------------------------- kernel ---
def build_kernel(ctx: ExitStack, tc: "tile.TileContext", outs, ins):
    nc = tc.nc
    y = outs[0]                       # [512,1024] f32 DRAM
    x, pred, wpack = ins              # wpack [128, NW*128] f32 DRAM

    consts = build_host_consts()
    wnames = sorted(consts.keys())

    sb = ctx.enter_context(tc.tile_pool(name="sb", bufs=1))
    sbR = ctx.enter_context(tc.tile_pool(name="sbR", bufs=2))
    wpool = ctx.enter_context(tc.tile_pool(name="wp", bufs=1))
    psB = ctx.enter_context(tc.tile_pool(name="psB", bufs=2, space="PSUM"))
    psY = ctx.enter_context(tc.tile_pool(name="psY", bufs=4, space="PSUM"))

    # ---- load + prepare weights ----
    wstage = sb.tile([P, len(wnames) * P], DT.float32, tag="wstage")
    nc.sync.dma_start(wstage[:], wpack[:, :len(wnames) * P])
    wt = {}
    BF16_W = {'T_mid', 'T_top', 'T_bot', 'T_up', 'T_dn', 'I'}
    for i, name in enumerate(wnames):
        if name == 'BVEC':
            continue
        src = wstage[:, i * P:(i + 1) * P]
        dt_w = DT.bfloat16 if name in BF16_W else DT.float32r
        t = wpool.tile([P, P], dt_w, name=f"w_{name}", tag=f"w_{name}")
        nc.vector.tensor_copy(t[:], src)
        wt[name] = t
    # fp32r variants of vertical matrices for the value path
    for name in ('T_mid', 'T_top', 'T_bot', 'T_up', 'T_dn'):
        t = wpool.tile([P, P], DT.float32r, name=f"wr_{name}", tag=f"wr_{name}")
        i = wnames.index(name)
        nc.vector.tensor_copy(t[:], wstage[:, i * P:(i + 1) * P])
        wt['R' + name[1:]] = t

    def TRv(c):
        return wt['T_top'] if c == 0 else (wt['T_bot'] if c == CH - 1 else wt['T_mid'])

    def Rv(c):
        return wt['R_top'] if c == 0 else (wt['R_bot'] if c == CH - 1 else wt['R_mid'])

    # ---- const bias vectors ----
    def make_const(val, tag):
        t = sb.tile([P, 1], DT.float32, tag=tag)
        nc.vector.memset(t[:], val)
        return t

    b_enc_max = make_const(-9.0 * ALPHA, "b_enc_max")
    b_enc_min = make_const(+9.0 * ALPHA, "b_enc_min")
    bv_mid = make_const(-4.0, "bv_mid")
    ib = wnames.index('BVEC')
    bv_top = sb.tile([P, 1], DT.float32, name="bv_top", tag="bv_top")
    nc.vector.tensor_copy(bv_top[:], wstage[:, ib * P:ib * P + 1])
    bv_bot = sb.tile([P, 1], DT.float32, name="bv_bot", tag="bv_bot")
    nc.vector.tensor_copy(bv_bot[:], wstage[:, ib * P + 1:ib * P + 2])
    one_c = make_const(1.0, "one_c")

    def bv(c):
        return bv_top if c == 0 else (bv_bot if c == CH - 1 else bv_mid)

    # ---- persistent image buffers ----
    lab = [sb.tile([P, W], DT.int32, name=f"lab{c}", tag=f"lab{c}") for c in range(CH)]
    OA = [sb.tile([P, W], DT.float32, name=f"OA{c}", tag=f"OA{c}") for c in range(CH)]
    OB = [sb.tile([P, W], DT.float32, name=f"OB{c}", tag=f"OB{c}") for c in range(CH)]
    for c in range(CH):
        nc.sync.dma_start(OA[c][:], x[c * P:(c + 1) * P, :])
        nc.sync.dma_start(lab[c][:], pred[c * P:(c + 1) * P, :])

    GW = W + 2

    def gtile(tag, dtype, guard_val, pool=sb):
        ts = [pool.tile([P, GW], dtype, name=f"{tag}{c}", tag=f"{tag}{c}") for c in range(CH)]
        for c in range(CH):
            for ap in (ts[c][:, 0:1], ts[c][:, GW - 1:GW]):
                if dtype == DT.float32r:
                    ap = ap.bitcast(DT.float32)
                nc.vector.memset(ap, guard_val)
        return ts

    Emax = gtile("Emax", DT.bfloat16, 0.0)
    Emin = gtile("Emin", DT.bfloat16, 0.0)
    m = [gtile(f"m{i}_", DT.bfloat16, 1.0) for i in range(4)]
    xm = gtile("xm", DT.float32r, 0.0)
    HN = [sb.tile([P, W], DT.bfloat16, name=f"HN{c}", tag=f"HMa{c}") for c in range(CH)]
    HMa = [sb.tile([P, W], DT.bfloat16, name=f"HMa{c}", tag=f"HMa{c}") for c in range(CH)]
    hlr = [sb.tile([P, W], DT.float32r, name=f"hlr{c}", tag=f"hlr{c}") for c in range(CH)]

    def data(t):
        return t[:, 1:W + 1]

    def shl(t):
        return t[:, 0:W]

    def shr(t):
        return t[:, 2:W + 2]

    def mm_group(pt, pairs):
        # split into N=512 sub-matmuls (PSUM bank limit); weight-major order
        # so consecutive matmuls share the stationary operand (fewer LDW).
        n = pt.shape[1]
        halves = list(range(0, n, 512))
        for i, (lhsT, rhs) in enumerate(pairs):
            for h0 in halves:
                nc.tensor.matmul(pt[:, h0:h0 + 512], lhsT,
                                 rhs[:, h0:h0 + 512], start=(i == 0),
                                 stop=(i == len(pairs) - 1))

    # ================= Phase M: encode + boundary masks ===================
    for c in range(CH):
        nc.scalar.activation(data(Emax[c]), lab[c][:], AF.Exp,
                             bias=b_enc_max[:], scale=ALPHA)
        nc.scalar.activation(data(Emin[c]), lab[c][:], AF.Exp,
                             bias=b_enc_min[:], scale=-ALPHA)
    for c in range(CH):
        nc.vector.tensor_tensor(HN[c][:], shl(Emin[c]), shr(Emin[c]), op=OP.add)
        nc.vector.tensor_tensor(HN[c][:], HN[c][:], data(Emin[c]), op=OP.add)
    for c in range(CH):
        p1 = psB.tile([P, W], DT.float32, name="pS1", tag="psb")
        pairs = [(wt['T_mid'][:], data(Emax[c])),
                 (wt['I'][:], shl(Emax[c])),
                 (wt['I'][:], shr(Emax[c]))]
        if c > 0:
            pairs.append((wt['T_up'][:], data(Emax[c - 1])))
        if c < CH - 1:
            pairs.append((wt['T_dn'][:], data(Emax[c + 1])))
        mm_group(p1[:], pairs)
        sc1 = sbR.tile([P, W], DT.bfloat16, name="sc1", tag="nb")
        nc.scalar.copy(sc1[:], p1[:])

        p2 = psB.tile([P, W], DT.float32, name="pS2", tag="psb")
        pairs = [(wt['T_mid'][:], HN[c][:])]
        if c > 0:
            pairs.append((wt['T_up'][:], HN[c - 1][:]))
        if c < CH - 1:
            pairs.append((wt['T_dn'][:], HN[c + 1][:]))
        mm_group(p2[:], pairs)
        pb = sbR.tile([P, W], DT.bfloat16, name="pb", tag="zt")
        nc.vector.tensor_tensor(pb[:], sc1[:], p2[:], op=OP.mult)
        nc.vector.tensor_scalar(data(m[3][c]), pb[:], PTHR, None, op0=OP.is_lt)

    # ================= Chain: m3 -> m2 -> m1 -> m0 ========================
    for k in range(3):
        mp, mn = m[3 - k], m[2 - k]
        for c in range(CH):
            ps = psB.tile([P, W], DT.float32, name="pCh", tag="psb")
            pairs = [(wt['T_mid'][:], data(mp[c])),
                     (wt['I'][:], shl(mp[c])),
                     (wt['I'][:], shr(mp[c]))]
            if c > 0:
                pairs.append((wt['T_up'][:], data(mp[c - 1])))
            if c < CH - 1:
                pairs.append((wt['T_dn'][:], data(mp[c + 1])))
            mm_group(ps[:], pairs)
            nc.scalar.activation(data(mn[c]), ps[:], AF.Relu, bias=bv(c)[:],
                                 scale=1.0)

    # ================= U loop =============================================
    cur, nxt = OA, OB
    for it in range(4):
        mi = m[it]
        for c in range(CH):
            nc.gpsimd.tensor_tensor(xm[c][:, 1:W + 1], cur[c][:], data(mi[c]),
                                    op=OP.mult)
            nc.gpsimd.tensor_tensor(HMa[c][:], shl(mi[c]), shr(mi[c]), op=OP.add)
        for c in range(CH):
            # HMa := full hsum3_rep(m) = mL + mR + m, with edge fixes
            nc.vector.tensor_tensor(HMa[c][:], HMa[c][:], data(mi[c]), op=OP.add)
            nc.vector.tensor_scalar(HMa[c][:, 0:1], mi[c][:, 1:2], 2.0, None,
                                    op0=OP.mult)
            nc.vector.tensor_tensor(HMa[c][:, 0:1], HMa[c][:, 0:1],
                                    mi[c][:, 2:3], op=OP.add)
            nc.vector.tensor_scalar(HMa[c][:, W - 1:W], mi[c][:, W:W + 1], 2.0,
                                    None, op0=OP.mult)
            nc.vector.tensor_tensor(HMa[c][:, W - 1:W], HMa[c][:, W - 1:W],
                                    mi[c][:, W - 1:W], op=OP.add)
            # hlr := xmL + xmR (DVE), edge fixes, then SH := hlr + xm (gpsimd)
            nc.vector.tensor_tensor(hlr[c][:], shl(xm[c]), shr(xm[c]), op=OP.add)
            nc.vector.tensor_tensor(hlr[c][:, 0:1], hlr[c][:, 0:1],
                                    xm[c][:, 1:2], op=OP.add)
            nc.vector.tensor_tensor(hlr[c][:, W - 1:W], hlr[c][:, W - 1:W],
                                    xm[c][:, W:W + 1], op=OP.add)
        for c in range(CH):
            nc.gpsimd.tensor_tensor(hlr[c][:], hlr[c][:], xm[c][:, 1:W + 1],
                                    op=OP.add)
        for c in range(CH):
            pn = psB.tile([P, W], DT.float32, name="pN", tag="psb")
            pairs = [(TRv(c)[:], HMa[c][:])]
            if c > 0:
                pairs.append((wt['T_up'][:], HMa[c - 1][:]))
            if c < CH - 1:
                pairs.append((wt['T_dn'][:], HMa[c + 1][:]))
            mm_group(pn[:], pairs)
            zt = sbR.tile([P, W], DT.bfloat16, name="zt", tag="zt")
            nc.scalar.activation(zt[:], pn[:], AF.Relu, bias=one_c[:],
                                 scale=-1.0)
            nb = sbR.tile([P, W], DT.float32, name="nb", tag="nb")
            nc.vector.reciprocal(nb[:], pn[:])
            Mk = sbR.tile([P, W], DT.int16, name="Mk", tag="Mk")
            nc.vector.tensor_tensor(Mk[:], data(mi[c]), zt[:], op=OP.add)

            for h in range(2):
                s = slice(h * 512, (h + 1) * 512)
                sg = slice(1 + h * 512, 1 + (h + 1) * 512)
                pyt = psY.tile([P, 512], DT.float32, name="pY", tag="psy")
                pairs = [(Rv(c)[:], hlr[c][:, s])]
                if c > 0:
                    pairs.append((wt['R_up'][:], hlr[c - 1][:, s]))
                if c < CH - 1:
                    pairs.append((wt['R_dn'][:], hlr[c + 1][:, s]))
                mm_group(pyt[:], pairs)
                nc.vector.tensor_tensor(nxt[c][:, s], pyt[:], nb[:, s],
                                        op=OP.mult)
            nc.vector.copy_predicated(nxt[c][:], Mk[:], cur[c][:])
        cur, nxt = nxt, cur

    # ================= Gaussian ==========================================
    GA = 18
    gs = [sb.tile([P, W + 2 * GA], DT.float32r, name=f"gs{c}", tag=f"lab{c}")
          for c in range(CH)]
    hg = [sb.tile([P, W], DT.float32r, name=f"Emin{c}", tag=f"Emin{c}") for c in range(CH)]
    yo = [sb.tile([P, W], DT.float32, name=f"Emax{c}", tag=f"Emax{c}") for c in range(CH)]
    for c in range(CH):
        nc.vector.tensor_copy(gs[c][:, GA:GA + W], cur[c][:])
        nc.vector.tensor_copy(gs[c][:, 0:GA],
                              cur[c][:, 0:1].to_broadcast((P, GA)))
        nc.vector.tensor_copy(gs[c][:, GA + W:],
                              cur[c][:, W - 1:W].to_broadcast((P, GA)))
    for c in range(CH):
        for h in range(2):
            ph = psY.tile([P, 512], DT.float32, name="pH", tag="psy")
            for j in range(7):
                off = GA + 6 * (j - 3) + h * 512
                nc.tensor.matmul(ph[:], wt[f'G{j}'][:], gs[c][:, off:off + 512],
                                 start=(j == 0), stop=(j == 6))
            nc.scalar.copy(hg[c][:, h * 512:(h + 1) * 512], ph[:])
    for c in range(CH):
        for h in range(2):
            s = slice(h * 512, (h + 1) * 512)
            pv = psY.tile([P, 512], DT.float32, name="pV", tag="psy")
            srcs = [cc for cc in range(CH) if f'B_{c}_{cc}' in wt]
            for i, cc in enumerate(srcs):
                nc.tensor.matmul(pv[:], wt[f'B_{c}_{cc}'][:], hg[cc][:, s],
                                 start=(i == 0), stop=(i == len(srcs) - 1))
            nc.scalar.copy(yo[c][:, s], pv[:])
    for c in range(CH):
        nc.sync.dma_start(y[c * P:(c + 1) * P, :], yo[c][:])


# ------------------------------------------------------------ host driver ---
_CACHE = {}


def _build_program():
    if 'nc' in _CACHE:
        return _CACHE['nc'], _CACHE['wpack']
    consts = build_host_consts()
    wnames = sorted(consts.keys())
    wpack = np.zeros((P, len(wnames) * P), np.float32)
    for i, n in enumerate(wnames):
        wpack[:, i * P:(i + 1) * P] = consts[n]

    nc = bacc.Bacc("TRN2", target_bir_lowering=False, debug=False,
                   num_devices=B)
    x_d = nc.dram_tensor("x", [H, W], DT.float32, kind="ExternalInput").ap()
    p_d = nc.dram_tensor("prediction", [H, W], DT.int32,
                         kind="ExternalInput").ap()
    w_d = nc.dram_tensor("wpack", list(wpack.shape), DT.float32,
                         kind="ExternalInput").ap()
    y_d = nc.dram_tensor("y", [H, W], DT.float32, kind="ExternalOutput").ap()
    with tile.TileContext(nc) as tc:
        with ExitStack() as ctx:
            build_kernel(ctx, tc, [y_d], [x_d, p_d, w_d])
    nc.compile()
    _CACHE['nc'] = nc
    _CACHE['wpack'] = wpack
    return nc, wpack


def _run(x, prediction, trace=False):
    from concourse.bass_utils import run_bass_kernel_spmd
    nc, wpack = _build_program()
    in_maps = []
    for i in range(B):
        in_maps.append({
            "x": np.ascontiguousarray(x[i, 0]).astype(np.float32),
            "prediction": np.ascontiguousarray(prediction[i, 0]).astype(np.int32),
            "wpack": wpack,
        })
    res = run_bass_kernel_spmd(nc, in_maps, core_ids=list(range(B)),
                               trace=trace)
    if trace:
        print(f"HW exec time: {res.exec_time_ns} ns "
              f"(mean {res.mean_exec_time_ns} ns, "
              f"slowest core {res.max_exec_time_core_id})")
        if res.instructions_and_trace:
            print("trace:", res.instructions_and_trace[1])
    out = np.stack([res.results[i]["y"] for i in range(B)], axis=0)
    return out[:, None, :, :].astype(np.float32)


def kernel(x: np.ndarray, prediction: np.ndarray) -> np.ndarray:
    return _run(x, prediction, trace=False)


def kernel_traced(x, prediction, trace=True):
    return _run(x, prediction, trace=trace)


if __name__ == "__main__":
    xs = np.random.randn(B, 1, H, W).astype(np.float32)
    ps = np.random.randint(0, 19, size=(B, 1, H, W)).astype(np.int32)
    print(kernel(xs, ps).shape)



# revision 8
# speedup vs baseline: 1.3692x; 1.3692x over previous
"""Trainium2 Bass kernel for nn_BoundarySuppressionWithSmoothing.

Full inputs: x [8,1,512,1024] f32, prediction [8,1,512,1024] int32.
Sharding: pure data parallel, image i -> core i.

v2: fp16 value path / bf16 encode path, weights baked into the NEFF as
Const tensors (no wpack input), Pool/DVE/Act engine rebalance, merged
[hm|hx] PSUM groups with a sign trick that folds the n==0 / m==1
selection into one divide + one predicated copy.

Per-core algorithm (image I [512,1024], 4 row-chunks of [128,1024]):
  - boundary detection via exp-encoded morphology (product compare)
  - masks m3..m0 via a mask-carried dilation chain
  - 4 iterations of masked 3x3 box average with replication padding
  - separable dilated 7x7 Gaussian (dilation 6) via PE banded matmuls
"""
import math
import sys
from contextlib import ExitStack

import numpy as np
import ml_dtypes

sys.path.insert(0, '/opt/trn_rl_repo')

import concourse.bass as bass  # noqa: E402
import concourse.bacc as bacc  # noqa: E402
import concourse.tile as tile  # noqa: E402
from concourse import mybir  # noqa: E402

P = 128
W = 1024
H = 512
CH = 4          # row chunks
B = 8           # batch == cores
GW = W + 2      # guarded width
ALPHA = 4.6     # morphology exp-encoding scale
PTHR = float(np.exp(4.2))   # product threshold for boundary test
KSEL = 512.0    # select-fold constant: v = KSEL*m - n
DT = mybir.dt
AF = mybir.ActivationFunctionType
OP = mybir.AluOpType


# ---------------------------------------------------------------- weights ---
def _gauss1d():
    size, sigma = 7, 1.0
    u = np.exp(-((np.arange(size) - 3.0) ** 2) / (2 * sigma ** 2))
    return (u / u.sum()).astype(np.float64)


def _tri():
    t = np.zeros((P, P), np.float64)
    for k in range(P):
        for d in (-1, 0, 1):
            if 0 <= k + d < P:
                t[k, k + d] = 1.0
    return t


BNAMES = ['bT_mid', 'bT_up', 'bT_dn', 'bI']
FNAMES = ['T_mid', 'T_up', 'T_dn', 'I',
          'nT_top', 'nT_mid', 'nT_bot', 'nT_up', 'nT_dn', 'KI',
          'G0', 'G1', 'G2', 'G3',
          'B_0_0', 'B_0_1', 'B_1_0', 'B_1_1', 'B_1_2', 'B_2_1', 'B_2_2',
          'B_2_3', 'B_3_2', 'B_3_3']


def build_host_consts():
    """Weight packs: wb [128, 4*128] bf16, wh [128, 24*128] fp16,
    bvec [128, 2] f32."""
    tri = _tri()
    t_up = np.zeros((P, P), np.float64); t_up[P - 1, 0] = 1.0
    t_dn = np.zeros((P, P), np.float64); t_dn[0, P - 1] = 1.0
    eye = np.eye(P)
    t_top = tri.copy(); t_top[0, 0] = 2.0
    t_bot = tri.copy(); t_bot[P - 1, P - 1] = 2.0

    c = {}
    c['bT_mid'], c['bT_up'], c['bT_dn'], c['bI'] = tri, t_up, t_dn, eye
    c['T_mid'], c['T_up'], c['T_dn'], c['I'] = tri, t_up, t_dn, eye
    c['nT_top'], c['nT_mid'], c['nT_bot'] = -t_top, -tri, -t_bot
    c['nT_up'], c['nT_dn'] = -t_up, -t_dn
    c['KI'] = KSEL * eye
    g = _gauss1d()
    for j in range(4):
        c[f'G{j}'] = eye * g[j]
    Wv = np.zeros((H, H), np.float64)
    for R in range(H):
        for j in range(7):
            S = min(max(R + 6 * (j - 3), 0), H - 1)
            Wv[R, S] += g[j]
    for cd in range(CH):
        for cs in range(CH):
            if abs(cd - cs) > 1:
                continue
            blk = Wv[cd * P:(cd + 1) * P, cs * P:(cs + 1) * P]
            c[f'B_{cd}_{cs}'] = np.ascontiguousarray(blk.T)

    wb = np.zeros((P, len(BNAMES) * P), np.float32)
    for i, n in enumerate(BNAMES):
        wb[:, i * P:(i + 1) * P] = c[n]
    wh = np.zeros((P, len(FNAMES) * P), np.float32)
    for i, n in enumerate(FNAMES):
        wh[:, i * P:(i + 1) * P] = c[n]
    bvec = np.full((P, 2), -4.0, np.float32)
    bvec[0, 0] = -3.0      # bv_top
    bvec[P - 1, 1] = -3.0  # bv_bot
    return (wb.astype(ml_dtypes.bfloat16), wh.astype(np.float16), bvec)


# ----------------------------------------------------------------- kernel ---
def build_kernel(ctx: ExitStack, tc: "tile.TileContext", outs, ins):
    nc = tc.nc
    y = outs[0]                       # [512,1024] fp16 DRAM
    x, pred = ins                     # x fp16, pred int8

    wb_np, wh_np, bvec_np = build_host_consts()
    wb_d = nc.inline_tensor(wb_np, name="wb").ap()
    wh_d = nc.inline_tensor(wh_np, name="wh").ap()
    bv_d = nc.inline_tensor(bvec_np, name="bvec").ap()

    sb = ctx.enter_context(tc.tile_pool(name="sb", bufs=1))
    sr = ctx.enter_context(tc.tile_pool(name="sr", bufs=3))
    ps = ctx.enter_context(tc.tile_pool(name="ps", bufs=2, space="PSUM"))

    # ---- weights ----
    wBt = sb.tile([P, len(BNAMES) * P], DT.bfloat16, name="wBt", tag="wB")
    nc.sync.dma_start(wBt[:], wb_d[:, :])
    wHt = sb.tile([P, len(FNAMES) * P], DT.float16, name="wHt", tag="wH")
    nc.sync.dma_start(wHt[:], wh_d[:, :])
    bvt = sb.tile([P, 2], DT.float32, name="bvt", tag="bv")
    nc.sync.dma_start(bvt[:], bv_d[:, :])

    def WB(name):
        i = BNAMES.index(name)
        return wBt[:, i * P:(i + 1) * P]

    def WH(name):
        i = FNAMES.index(name)
        return wHt[:, i * P:(i + 1) * P]

    def nT(c):
        return WH('nT_top') if c == 0 else (
            WH('nT_bot') if c == CH - 1 else WH('nT_mid'))

    bv_mid = sb.tile([P, 1], DT.float32, name="bv_mid", tag="bvm")
    nc.vector.memset(bv_mid[:], -4.0)
    b_enc_max = sb.tile([P, 1], DT.float32, name="b_enc_max", tag="bem")
    nc.vector.memset(b_enc_max[:], -9.0 * ALPHA)
    b_enc_min = sb.tile([P, 1], DT.float32, name="b_enc_min", tag="ben")
    nc.vector.memset(b_enc_min[:], 9.0 * ALPHA)

    def bv(c):
        return bvt[:, 0:1] if c == 0 else (
            bvt[:, 1:2] if c == CH - 1 else bv_mid[:])

    # ---- persistent tiles ----
    lab = [sb.tile([P, W], DT.int8, name=f"lab{c}", tag=f"lab{c}") for c in range(CH)]
    OA = [sb.tile([P, GW], DT.float16, name=f"OA{c}", tag=f"OA{c}") for c in range(CH)]
    OB = [sb.tile([P, GW], DT.float16, name=f"OB{c}", tag=f"OB{c}") for c in range(CH)]
    for c in range(CH):
        nc.sync.dma_start(OA[c][:, 1:W + 1], x[c * P:(c + 1) * P, :])
        nc.sync.dma_start(lab[c][:], pred[c * P:(c + 1) * P, :])
    for c in range(CH):
        nc.scalar.copy(OA[c][:, 0:1], OA[c][:, 1:2])
        nc.scalar.copy(OA[c][:, W + 1:W + 2], OA[c][:, W:W + 1])

    # encode tiles, guards = 0 (geodesic border)
    EM = [sb.tile([P, GW], DT.bfloat16, name=f"EM{c}", tag=f"EM{c}") for c in range(CH)]
    EN = [sb.tile([P, GW], DT.bfloat16, name=f"EN{c}", tag=f"EN{c}") for c in range(CH)]
    for c in range(CH):
        for t in (EM[c], EN[c]):
            nc.vector.memset(t[:, 0:1], 0.0)
            nc.vector.memset(t[:, GW - 1:GW], 0.0)
    HN = [sb.tile([P, W], DT.bfloat16, name=f"HN{c}", tag=f"HN{c}") for c in range(CH)]

    # mask tiles: [mask(GW) | xm(GW)], mask guards = 1 (geodesic)
    M = [[sb.tile([P, 2 * GW], DT.float16, name=f"m{i}_{c}", tag=f"m{i}_{c}") for c in range(CH)]
         for i in range(4)]
    for i in range(4):
        for c in range(CH):
            nc.vector.memset(M[i][c][:, 0:1], 1.0)
            nc.vector.memset(M[i][c][:, GW - 1:GW], 1.0)
    comb = [sb.tile([P, 2 * W], DT.float16, name=f"comb{c}", tag=f"comb{c}") for c in range(CH)]

    def data(t):
        return t[:, 1:W + 1]

    def shl(t):
        return t[:, 0:W]

    def shr(t):
        return t[:, 2:W + 2]

    def flip_guards(t):
        # mask guards: geodesic (1.0) -> replication (edge value)
        nc.scalar.copy(t[:, 0:1], t[:, 1:2])
        nc.scalar.copy(t[:, GW - 1:GW], t[:, W:W + 1])

    def mm_spans(pt, items):
        """items: list of (lhsT, rhs_ap, lo, hi); accumulate rhs into
        pt[:, lo:hi] split at 512. Pair-major for LDW sharing."""
        first = {}
        last = {}
        for idx, (_, _, lo, hi) in enumerate(items):
            for h0 in range(lo, hi, 512):
                first.setdefault(h0, idx)
                last[h0] = idx
        for idx, (lhsT, rhs, lo, hi) in enumerate(items):
            for k, h0 in enumerate(range(lo, hi, 512)):
                nc.tensor.matmul(pt[:, h0:h0 + 512], lhsT,
                                 rhs[:, k * 512:(k + 1) * 512],
                                 start=(first[h0] == idx),
                                 stop=(last[h0] == idx))

    # ================= Phase M: encode + boundary mask m3 =================
    for c in range(CH):
        nc.scalar.activation(data(EM[c]), lab[c][:], AF.Exp,
                             bias=b_enc_max[:], scale=ALPHA)
        nc.scalar.activation(data(EN[c]), lab[c][:], AF.Exp,
                             bias=b_enc_min[:], scale=-ALPHA)
    for c in range(CH):
        tn = sr.tile([P, W], DT.bfloat16, name="tn", tag="sc1")
        nc.vector.tensor_tensor(tn[:], shl(EN[c]), shr(EN[c]), op=OP.add)
        nc.vector.tensor_tensor(HN[c][:], tn[:], data(EN[c]), op=OP.add)
    for c in range(CH):
        p1 = ps.tile([P, 2 * W], DT.float32, name="p1", tag="psb")
        items = [(WB('bT_mid'), data(EM[c]), 0, W),
                 (WB('bI'), shl(EM[c]), 0, W),
                 (WB('bI'), shr(EM[c]), 0, W)]
        if c > 0:
            items.append((WB('bT_up'), data(EM[c - 1]), 0, W))
        if c < CH - 1:
            items.append((WB('bT_dn'), data(EM[c + 1]), 0, W))
        mm_spans(p1[:], items)
        sc1 = sr.tile([P, W], DT.bfloat16, name="sc1", tag="sc1")
        nc.scalar.copy(sc1[:], p1[:, 0:W])

        p2 = ps.tile([P, 2 * W], DT.float32, name="p2", tag="psb")
        items = [(WB('bT_mid'), HN[c][:], 0, W)]
        if c > 0:
            items.append((WB('bT_up'), HN[c - 1][:], 0, W))
        if c < CH - 1:
            items.append((WB('bT_dn'), HN[c + 1][:], 0, W))
        mm_spans(p2[:], items)
        pb = sr.tile([P, W], DT.bfloat16, name="pb", tag="pb")
        nc.vector.tensor_tensor(pb[:], sc1[:], p2[:, 0:W], op=OP.mult)
        nc.vector.tensor_scalar(data(M[3][c]), pb[:], PTHR, None, op0=OP.is_lt)

    # ================= Chain: m3 -> m2 -> m1 -> m0 ========================
    for k in range(3):
        mp, mn = M[3 - k], M[2 - k]
        for c in range(CH):
            pc = ps.tile([P, 2 * W], DT.float32, name="pc", tag="psb")
            items = [(WH('T_mid'), data(mp[c]), 0, W),
                     (WH('I'), shl(mp[c]), 0, W),
                     (WH('I'), shr(mp[c]), 0, W)]
            if c > 0:
                items.append((WH('T_up'), data(mp[c - 1]), 0, W))
            if c < CH - 1:
                items.append((WH('T_dn'), data(mp[c + 1]), 0, W))
            mm_spans(pc[:], items)
            nc.scalar.activation(data(mn[c]), pc[:, 0:W], AF.Relu,
                                 bias=bv(c)[:], scale=1.0)
        for c in range(CH):
            flip_guards(mp[c])
    for c in range(CH):
        flip_guards(M[0][c])

    # ================= U loop =============================================
    cur, nxt = OA, OB
    for it in range(4):
        mi = M[it]
        for c in range(CH):
            # xm (with replication guards) into the second GW segment
            nc.gpsimd.tensor_tensor(mi[c][:, GW:2 * GW], cur[c][:, 0:GW],
                                    mi[c][:, 0:GW], op=OP.mult)
        for c in range(CH):
            t1 = sr.tile([P, 2 * GW - 2], DT.float16, name="t1", tag="t1")
            nc.vector.tensor_tensor(t1[:], mi[c][:, 0:2 * GW - 2],
                                    mi[c][:, 2:2 * GW], op=OP.add)
            nc.vector.tensor_tensor(comb[c][:, 0:W], t1[:, 0:W],
                                    mi[c][:, 1:W + 1], op=OP.add)
            nc.vector.tensor_tensor(comb[c][:, W:2 * W], t1[:, GW:GW + W],
                                    mi[c][:, GW + 1:GW + W + 1], op=OP.add)
        for c in range(CH):
            pn = ps.tile([P, 2 * W], DT.float32, name="pn", tag="psb")
            items = [(nT(c), comb[c][:], 0, 2 * W),
                     (WH('KI'), mi[c][:, 1:W + 1], 0, W)]
            if c > 0:
                items.append((WH('nT_up'), comb[c - 1][:], 0, 2 * W))
            if c < CH - 1:
                items.append((WH('nT_dn'), comb[c + 1][:], 0, 2 * W))
            mm_spans(pn[:], items)
            # v = K*m - n in pn[0:W]; S' = -sum3x3(xm) in pn[W:2W]
            # rden = 1/v: negative on the average path (m=0, n>0),
            # positive where cur must be kept (m=1 -> 1/(K-n); n=0 -> +inf)
            rden = sr.tile([P, W], DT.float16, name="rden", tag="den")
            with nc.allow_low_precision(reason="v is a small exact integer"):
                nc.vector.reciprocal(rden[:], pn[:, 0:W])
            Mk = sr.tile([P, W], DT.int16, name="Mk", tag="Mk")
            nc.vector.tensor_scalar(Mk[:], rden[:], 0.0, None, op0=OP.is_gt)
            sbS = sr.tile([P, W], DT.float16, name="sbS", tag="sbS")
            nc.scalar.copy(sbS[:], pn[:, W:2 * W])
            nc.gpsimd.tensor_tensor(data(nxt[c]), sbS[:], rden[:],
                                    op=OP.mult)
            nc.vector.copy_predicated(data(nxt[c]), Mk[:], data(cur[c]))
            nc.scalar.copy(nxt[c][:, 0:1], nxt[c][:, 1:2])
            nc.scalar.copy(nxt[c][:, W + 1:W + 2], nxt[c][:, W:W + 1])
        cur, nxt = nxt, cur

    # ================= Gaussian ==========================================
    GA = 18
    gs = [sb.tile([P, W + 2 * GA], DT.float16, name=f"gs{c}", tag=f"gs{c}") for c in range(CH)]
    hg = [sb.tile([P, W], DT.float16, name=f"hg{c}", tag=f"hg{c}") for c in range(CH)]
    yo = [sb.tile([P, W], DT.float16, name=f"yo{c}", tag=f"yo{c}") for c in range(CH)]
    for c in range(CH):
        nc.vector.tensor_copy(gs[c][:, GA:GA + W], data(cur[c]))
        nc.vector.tensor_copy(gs[c][:, 0:GA],
                              cur[c][:, 1:2].to_broadcast((P, GA)))
        nc.vector.tensor_copy(gs[c][:, GA + W:],
                              cur[c][:, W:W + 1].to_broadcast((P, GA)))
    taps = [(0, -18), (0, 18), (1, -12), (1, 12), (2, -6), (2, 6), (3, 0)]
    for c in range(CH):
        ph = ps.tile([P, 2 * W], DT.float32, name="ph", tag="psb")
        items = [(WH(f'G{gj}'), gs[c][:, GA + off:GA + off + W], 0, W)
                 for gj, off in taps]
        mm_spans(ph[:], items)
        nc.scalar.copy(hg[c][:], ph[:, 0:W])
    for c in range(CH):
        pv = ps.tile([P, 2 * W], DT.float32, name="pv", tag="psb")
        srcs = [cc for cc in range(CH) if abs(cc - c) <= 1]
        items = [(WH(f'B_{c}_{cc}'), hg[cc][:], 0, W) for cc in srcs]
        mm_spans(pv[:], items)
        nc.scalar.copy(yo[c][:], pv[:, 0:W])
    for c in range(CH):
        nc.sync.dma_start(y[c * P:(c + 1) * P, :], yo[c][:])


# ------------------------------------------------------------ host driver ---
_CACHE = {}


def _build_program():
    if 'nc' in _CACHE:
        return _CACHE['nc'], None
    nc = bacc.Bacc("TRN2", target_bir_lowering=False, debug=False,
                   num_devices=B)
    x_d = nc.dram_tensor("x", [H, W], DT.float16, kind="ExternalInput").ap()
    p_d = nc.dram_tensor("prediction", [H, W], DT.int8,
                         kind="ExternalInput").ap()
    y_d = nc.dram_tensor("y", [H, W], DT.float16, kind="ExternalOutput").ap()
    with tile.TileContext(nc) as tc:
        with ExitStack() as ctx:
            build_kernel(ctx, tc, [y_d], [x_d, p_d])
    nc.compile()
    _CACHE['nc'] = nc
    return nc, None


def _make_dispatch(nc):
    """Compiled fast-dispatch SPMD callable over 8 cores."""
    import jax
    from jax.sharding import Mesh, PartitionSpec
    from jax.experimental.shard_map import shard_map
    from concourse import bass2jax
    bass2jax.install_neuronx_cc_hook()

    partition_name = (nc.partition_id_tensor.name
                      if nc.partition_id_tensor else None)
    in_names, out_names, out_avals, zero_outs = [], [], [], []
    for alloc in nc.m.functions[0].allocations:
        if not isinstance(alloc, mybir.MemoryLocationSet):
            continue
        name = alloc.memorylocations[0].name
        if alloc.kind == "ExternalInput":
            if name != partition_name:
                in_names.append(name)
        elif alloc.kind == "ExternalOutput":
            out_names.append(name)
            shape = tuple(alloc.tensor_shape)
            dtype = mybir.dt.np(alloc.dtype)
            out_avals.append(jax.core.ShapedArray(shape, dtype))
            zero_outs.append(np.zeros(shape, dtype))
    n_params, n_outs = len(in_names), len(out_avals)
    in_names_all = list(in_names) + list(out_names)
    if partition_name is not None:
        in_names_all.append(partition_name)
    donate = tuple(range(n_params, n_params + n_outs))

    def _body(*args):
        operands = list(args)
        if partition_name is not None:
            operands.append(bass2jax.partition_id_tensor())
        outs = bass2jax._bass_exec_p.bind(
            *operands, out_avals=tuple(out_avals),
            in_names=tuple(in_names_all), out_names=tuple(out_names),
            lowering_input_output_aliases=(),
            sim_require_finite=True, sim_require_nnan=True, nc=nc)
        return tuple(outs)

    devices = jax.devices()[:B]
    mesh = Mesh(np.asarray(devices), ("core",))
    concat_shapes = [(B * s[0], *s[1:]) for s in
                     [tuple(a.shape) for a in
                      [jax.core.ShapedArray((H, W), np.float16),
                       jax.core.ShapedArray((H, W), np.int8)]]]
    del concat_shapes

    def compile_fn():
        jf = jax.jit(
            shard_map(_body, mesh=mesh,
                      in_specs=(PartitionSpec("core"),) * (n_params + n_outs),
                      out_specs=(PartitionSpec("core"),) * n_outs,
                      check_rep=False),
            donate_argnums=donate, keep_unused=True)
        abstract = [jax.core.ShapedArray((B * H, W), np.float16),
                    jax.core.ShapedArray((B * H, W), np.int8)]
        zabstract = [jax.core.ShapedArray((B * z.shape[0], *z.shape[1:]),
                                          z.dtype) for z in zero_outs]
        return jf.lower(*abstract, *zabstract).compile()

    sharded = bass2jax.fast_dispatch_compile(compile_fn)
    return sharded, in_names, out_names, zero_outs


def _run(x, prediction):
    nc, _ = _build_program()
    if 'dispatch' not in _CACHE:
        _CACHE['dispatch'] = _make_dispatch(nc)
    sharded, in_names, out_names, zero_outs = _CACHE['dispatch']
    assert in_names == ["x", "prediction"], in_names
    xc = np.ascontiguousarray(x[:, 0]).astype(np.float16).reshape(B * H, W)
    pc = np.ascontiguousarray(prediction[:, 0]).astype(np.int8).reshape(B * H, W)
    zc = [np.zeros((B * z.shape[0], *z.shape[1:]), z.dtype) for z in zero_outs]
    out_arrs = sharded(xc, pc, *zc)
    out = np.asarray(out_arrs[0]).reshape(B, H, W)
    return out[:, None, :, :].astype(np.float32)


def kernel(x: np.ndarray, prediction: np.ndarray) -> np.ndarray:
    return _run(x, prediction)


if __name__ == "__main__":
    xs = np.random.randn(B, 1, H, W).astype(np.float32)
    ps_ = np.random.randint(0, 19, size=(B, 1, H, W)).astype(np.int32)
    print(kernel(xs, ps_).shape)
